# revision 26
# baseline (speedup 1.0000x reference)
"""Distributed Trainium2 kernel for the AdaGCL VGAE view (8 NeuronCores).

Computation (reference):
    h      = tanh(spmm(A, x @ W1 + b1))
    mu     = spmm(A, h @ Wmu + bmu)
    logstd = spmm(A, h @ Wls + bls)
    z      = mu + eps * exp(logstd)
    adj    = z @ z.T
Returns (z, adj, mu, logstd).

Strategy: 1D row partition of nodes over 8 cores. The sparse adjacency
(E=262144 edges over N=8192 nodes) is densified on the host into per-core
A.T column blocks stored fp16 (resident in SBUF); spmm becomes TensorE
matmuls with fp32 PSUM accumulation. bf16 is numerically insufficient here
(logstd spans +-19, exp() amplifies absolute error; fp16's 11-bit mantissa
keeps the relative error ~2e-3). z stays fp32 and z @ z.T runs as a
bf16 hi/lo split (hi = bf16(z), lo = bf16(z - hi) stacked on the K axis)
so one K=128 bf16 matmul yields ~fp32 precision at full PE rate.

Communication: the three AllGathers
(pre-activations fp16, head pre-activations fp16, z^T fp32) are each split
into two half-node collectives; the contraction (j) axis is permuted
host-side into gather order (half major, then rank, then row) so compute
on the first half overlaps the second half's wire time.
"""

import sys
import numpy as np

sys.path.insert(0, "/opt/trn_rl_repo")

from concourse import bacc, mybir, tile  # noqa: E402
from concourse.bass_utils import run_bass_kernel_spmd  # noqa: E402

N = 8192
E = 262144
IN_DIM, HID_DIM, Z_DIM = 512, 256, 64
N_CORES = 8
P = N // N_CORES          # 1024 rows per core
HP = P // 2               # 512 rows per gather half
NT = N // 128             # 64 j-tiles of 128 nodes
PT = P // 128             # 8 i-tiles per core

F16 = mybir.dt.float16
F32 = mybir.dt.float32
BF16 = mybir.dt.bfloat16

_NC_CACHE = {}

# new-j -> old-j permutation of the contraction axis: jj = (h, r, o)
# with h in {0,1} half, r rank, o in [0, 512) -> old = r*1024 + h*512 + o.
_PERM = np.concatenate([
    r * P + h * HP + np.arange(HP)
    for h in range(2) for r in range(N_CORES)
])


def _build():
    nc = bacc.Bacc("TRN2", target_bir_lowering=False, debug=False,
                   num_devices=N_CORES)

    # ---- per-core external inputs ----
    xT = nc.dram_tensor("xT", [IN_DIM, P], F16, kind="ExternalInput")
    w1 = nc.dram_tensor("w1", [IN_DIM, HID_DIM], F16, kind="ExternalInput")
    b1 = nc.dram_tensor("b1", [1, HID_DIM], F16, kind="ExternalInput")
    wml = nc.dram_tensor("wml", [HID_DIM, 2 * Z_DIM], F16, kind="ExternalInput")
    bml = nc.dram_tensor("bml", [1, 2 * Z_DIM], F16, kind="ExternalInput")
    at = nc.dram_tensor("at", [N, P], F16, kind="ExternalInput")  # A.T, j perm'd
    epsT = nc.dram_tensor("epsT", [Z_DIM, P], F32, kind="ExternalInput")
    ident = nc.dram_tensor("ident", [128, 128], F32, kind="ExternalInput")

    # ---- per-core external outputs (row shards) ----
    z_out = nc.dram_tensor("z", [P, Z_DIM], F32, kind="ExternalOutput")
    adj_out = nc.dram_tensor("adj", [P, N], BF16, kind="ExternalOutput")
    mu_out = nc.dram_tensor("mu", [P, Z_DIM], F32, kind="ExternalOutput")
    ls_out = nc.dram_tensor("ls", [P, Z_DIM], F32, kind="ExternalOutput")

    # ---- collective bounce buffers ----
    ag1_in = [nc.dram_tensor(f"ag1_in{h}", [HP, HID_DIM], F16) for h in range(2)]
    ag1_out = [nc.dram_tensor(f"ag1_out{h}", [N_CORES * HP, HID_DIM], F16,
                              addr_space="Shared") for h in range(2)]
    ag2_in = [nc.dram_tensor(f"ag2_in{h}", [HP, 2 * Z_DIM], F16) for h in range(2)]
    ag2_out = [nc.dram_tensor(f"ag2_out{h}", [N_CORES * HP, 2 * Z_DIM], F16,
                              addr_space="Shared") for h in range(2)]
    # z^T gathered as bf16 hi/lo split stacked on the partition axis:
    # rows 0:64 = bf16(z^T), rows 64:128 = bf16(z^T - hi). One K=128 bf16
    # matmul then computes (hi+lo) @ (hi+lo)^T with fp32 accumulation --
    # full-rate PE at ~fp32 precision (rel err ~2^-16).
    ag3_in = [nc.dram_tensor(f"ag3_in{h}", [128, HP], BF16) for h in range(2)]
    ag3_out = [nc.dram_tensor(f"ag3_out{h}", [N_CORES * 128, HP], BF16,
                              addr_space="Shared") for h in range(2)]

    groups = [list(range(N_CORES))]

    def allgather(src, dst):
        nc.gpsimd.collective_compute(
            "AllGather", mybir.AluOpType.bypass, replica_groups=groups,
            ins=[src.ap().opt()], outs=[dst.ap().opt()])

    at_r = at.ap().rearrange("(t p) i -> p t i", p=128)        # [128, 64, 1024]
    xT_r = xT.ap().rearrange("(t p) i -> p t i", p=128)        # [128, 4, 1024]
    w1_r = w1.ap().rearrange("(t p) f -> p t f", p=128)        # [128, 4, 256]
    wml_r = wml.ap().rearrange("(t p) f -> p t f", p=128)      # [128, 2, 128]
    ag1_in_r = [a.ap().rearrange("(t p) f -> p t f", p=128) for a in ag1_in]
    ag2_in_r = [a.ap().rearrange("(t p) f -> p t f", p=128) for a in ag2_in]
    ag1_out_r = [a.ap().rearrange("(t p) f -> p t f", p=128) for a in ag1_out]
    ag2_out_r = [a.ap().rearrange("(t p) f -> p t f", p=128) for a in ag2_out]
    # adj columns in (rank, half, offset) original order vs new-j order
    adj_out_r = adj_out.ap().rearrange("(t p) (r h o) -> p t r h o",
                                       p=128, h=2, o=HP)
    z_out_r = z_out.ap().rearrange("(t p) f -> p t f", p=128)
    mu_out_r = mu_out.ap().rearrange("(t p) f -> p t f", p=128)
    ls_out_r = ls_out.ap().rearrange("(t p) f -> p t f", p=128)

    with tile.TileContext(nc) as tc:
        with tc.tile_pool(name="work", bufs=1) as work:
            ones = work.tile([1, 128], F16, tag="ones")
            nc.vector.memset(ones[:], 1.0)
            ident_sb = work.tile([128, 128], F32, tag="ident")
            nc.scalar.dma_start(out=ident_sb[:], in_=ident[:])

            with tc.tile_pool(name="res", bufs=1) as res:
                # A.T resident for both spmm passes: [128, 64, 1024] fp16.
                # gpsimd (SWDGE) queue so the sync queue serves the
                # latency-critical stage-1 inputs first.
                at_sb = res.tile([128, NT, P], F16)
                for c in range(8):
                    nc.gpsimd.dma_start(out=at_sb[:, 8 * c:8 * c + 8, :],
                                        in_=at_r[:, 8 * c:8 * c + 8, :])

                # ---- stage 1: pre_h = x @ W1 + b1 (per-core rows) ----
                with tc.tile_pool(name="s1", bufs=1) as s1, \
                     tc.tile_pool(name="ps1", bufs=4, space="PSUM") as ps1:
                    xT_sb = s1.tile([128, IN_DIM // 128, P], F16)
                    w1_sb = s1.tile([128, IN_DIM // 128, HID_DIM], F16)
                    b1_sb = s1.tile([1, HID_DIM], F16)
                    nc.sync.dma_start(out=xT_sb[:], in_=xT_r[:])
                    nc.sync.dma_start(out=w1_sb[:], in_=w1_r[:])
                    nc.sync.dma_start(out=b1_sb[:], in_=b1[:])
                    for h in range(2):
                        for lt in range(PT // 2):
                            it = h * (PT // 2) + lt
                            ph = ps1.tile([128, HID_DIM], F32)
                            for kt in range(IN_DIM // 128):
                                nc.tensor.matmul(
                                    ph[:],
                                    xT_sb[:, kt, it * 128:(it + 1) * 128],
                                    w1_sb[:, kt, :],
                                    start=(kt == 0), stop=False)
                            nc.tensor.matmul(ph[:], ones[:], b1_sb[:],
                                             start=False, stop=True)
                            ph_sb = s1.tile([128, HID_DIM], F16, tag="ph_sb",
                                            bufs=4)
                            if it % 2 == 0:
                                nc.scalar.copy(ph_sb[:], ph[:])
                            else:
                                nc.vector.tensor_copy(ph_sb[:], ph[:])
                            nc.sync.dma_start(out=ag1_in_r[h][:, lt, :],
                                              in_=ph_sb[:])
                        allgather(ag1_in[h], ag1_out[h])

                # ---- stage 3 + heads, pipelined by output half ----
                # The A @ pre_h accumulation runs once per i-half; as soon as
                # half 0's tanh + head projections are out, AG2a flies while
                # the PE grinds half 1.
                hT_sb = work.tile([128, HID_DIM // 128, P], F16, tag="hT")
                with tc.tile_pool(name="s3", bufs=1) as s3, \
                     tc.tile_pool(name="ps3", bufs=1, space="PSUM") as ps3, \
                     tc.tile_pool(name="ps4", bufs=4, space="PSUM") as ps4:
                    preh_sb = s3.tile([128, NT, HID_DIM], F16)
                    wml_sb = s3.tile([128, HID_DIM // 128, 2 * Z_DIM], F16)
                    bml_sb = s3.tile([1, 2 * Z_DIM], F16)
                    nc.sync.dma_start(out=wml_sb[:], in_=wml_r[:])
                    nc.sync.dma_start(out=bml_sb[:], in_=bml[:])
                    for h in range(2):
                        for c in range(4):
                            eng = nc.sync if c % 2 == 0 else nc.gpsimd
                            o = h * 32 + 8 * c
                            eng.dma_start(out=preh_sb[:, o:o + 8, :],
                                          in_=ag1_out_r[h][:, 8 * c:8 * c + 8, :])
                    hps = [[ps3.tile([128, 512], F32, name=f"hps{ih}{i}",
                                     tag=f"hps{ih}{i}") for i in range(2)]
                           for ih in range(2)]
                    # j-tiles 0..31 for all four groups first (AG1b is still
                    # in flight); then finish each i-half's j 32..63 and fire
                    # its downstream work immediately.
                    for jt in range(NT // 2):
                        for ft in range(2):
                            for ih in range(2):
                                nc.tensor.matmul(
                                    hps[ih][ft][:],
                                    preh_sb[:, jt, ft * 128:(ft + 1) * 128],
                                    at_sb[:, jt, ih * 512:(ih + 1) * 512],
                                    start=(jt == 0), stop=False)
                    for ih in range(2):
                        for jt in range(NT // 2, NT):
                            for ft in range(2):
                                nc.tensor.matmul(
                                    hps[ih][ft][:],
                                    preh_sb[:, jt, ft * 128:(ft + 1) * 128],
                                    at_sb[:, jt, ih * 512:(ih + 1) * 512],
                                    start=False, stop=(jt == NT - 1))
                        for ft in range(2):
                            nc.scalar.activation(
                                hT_sb[:, ft, ih * 512:(ih + 1) * 512],
                                hps[ih][ft][:],
                                mybir.ActivationFunctionType.Tanh)
                        for lt in range(PT // 2):
                            it = ih * (PT // 2) + lt
                            mlp = ps4.tile([128, 2 * Z_DIM], F32)
                            for ft in range(HID_DIM // 128):
                                nc.tensor.matmul(
                                    mlp[:],
                                    hT_sb[:, ft, it * 128:(it + 1) * 128],
                                    wml_sb[:, ft, :],
                                    start=(ft == 0), stop=False)
                            nc.tensor.matmul(mlp[:], ones[:], bml_sb[:],
                                             start=False, stop=True)
                            ml_sb = s3.tile([128, 2 * Z_DIM], F16, tag="ml_sb",
                                            bufs=4)
                            if it % 2 == 0:
                                nc.scalar.copy(ml_sb[:], mlp[:])
                            else:
                                nc.vector.tensor_copy(ml_sb[:], mlp[:])
                            nc.sync.dma_start(out=ag2_in_r[ih][:, lt, :],
                                              in_=ml_sb[:])
                        allgather(ag2_in[ih], ag2_out[ih])

                # ---- stage 5 + reparameterize, pipelined by i-half ----
                # Each i-half: accumulate (A @ ml).T, then immediately build
                # z^T hi/lo for that half and fire its AllGather while the
                # PE grinds the other half.
                mlT_sb = work.tile([128, P], F32, tag="mlT")
                zT_sb = work.tile([Z_DIM, P], F32, tag="zT")
                with tc.tile_pool(name="s5", bufs=1) as s5, \
                     tc.tile_pool(name="ps5", bufs=1, space="PSUM") as ps5:
                    mlf_sb = s5.tile([128, NT, 2 * Z_DIM], F16)
                    for h in range(2):
                        for c in range(2):
                            eng = nc.sync if c % 2 == 0 else nc.gpsimd
                            o = h * 32 + 16 * c
                            eng.dma_start(
                                out=mlf_sb[:, o:o + 16, :],
                                in_=ag2_out_r[h][:, 16 * c:16 * c + 16, :])
                    epsT_sb = s5.tile([Z_DIM, P], F32, tag="epsT")
                    nc.sync.dma_start(out=epsT_sb[:], in_=epsT[:])
                    expT_hi = s5.tile([128, P], F32, tag="expT_hi")
                    expT_lo = s5.tile([Z_DIM, P], F32, tag="expT_lo")
                    zhi = s5.tile([Z_DIM, P], BF16, tag="zhi")
                    zlo = s5.tile([Z_DIM, P], BF16, tag="zlo")
                    mps = [ps5.tile([128, 512], F32, name=f"mps{ih}",
                                    tag=f"mps{ih}") for ih in range(2)]
                    for jt in range(NT // 2):
                        for ih in range(2):
                            nc.tensor.matmul(
                                mps[ih][:],
                                mlf_sb[:, jt, :],
                                at_sb[:, jt, ih * 512:(ih + 1) * 512],
                                start=(jt == 0), stop=False)
                    for ih in range(2):
                        for jt in range(NT // 2, NT):
                            nc.tensor.matmul(
                                mps[ih][:],
                                mlf_sb[:, jt, :],
                                at_sb[:, jt, ih * 512:(ih + 1) * 512],
                                start=False, stop=(jt == NT - 1))
                        sl = slice(ih * HP, (ih + 1) * HP)
                        nc.scalar.copy(mlT_sb[:, sl], mps[ih][:])
                        nc.scalar.activation(expT_hi[64:128, sl],
                                             mlT_sb[64:128, sl],
                                             mybir.ActivationFunctionType.Exp)
                        nc.sync.dma_start(out=expT_lo[:, sl],
                                          in_=expT_hi[64:128, sl])
                        nc.vector.tensor_mul(zT_sb[:, sl], epsT_sb[:, sl],
                                             expT_lo[:, sl])
                        nc.vector.tensor_add(zT_sb[:, sl], zT_sb[:, sl],
                                             mlT_sb[0:64, sl])
                        nc.vector.tensor_copy(zhi[:, sl], zT_sb[:, sl])
                        nc.vector.tensor_sub(zlo[:, sl], zT_sb[:, sl],
                                             zhi[:, sl])
                        nc.sync.dma_start(out=ag3_in[ih][0:Z_DIM, :],
                                          in_=zhi[:, sl])
                        nc.sync.dma_start(out=ag3_in[ih][Z_DIM:128, :],
                                          in_=zlo[:, sl])
                        allgather(ag3_in[ih], ag3_out[ih])

                    # natural-layout outputs mu / logstd / z via PE transposes
                    with tc.tile_pool(name="ps6", bufs=2, space="PSUM") as ps6:
                        mlnat = s5.tile([128, PT, 2 * Z_DIM], F32, tag="mlnat")
                        znat = s5.tile([128, PT, Z_DIM], F32, tag="znat")
                        for it in range(PT):
                            tp = ps6.tile([128, 128], F32, tag="tp")
                            nc.tensor.transpose(
                                tp[:], mlT_sb[:, it * 128:(it + 1) * 128],
                                ident_sb[:])
                            nc.vector.tensor_copy(mlnat[:, it, :], tp[:])
                            tz = ps6.tile([128, Z_DIM], F32, tag="tz")
                            nc.tensor.transpose(
                                tz[:], zT_sb[:, it * 128:(it + 1) * 128],
                                ident_sb[0:64, 0:64])
                            nc.vector.tensor_copy(znat[:, it, :], tz[:])
                        nc.sync.dma_start(out=mu_out_r[:],
                                          in_=mlnat[:, :, 0:Z_DIM])
                        nc.sync.dma_start(out=ls_out_r[:],
                                          in_=mlnat[:, :, Z_DIM:2 * Z_DIM])
                        nc.sync.dma_start(out=z_out_r[:], in_=znat[:])

            # ---- stage 7: adj = z_shard @ z_full.T (A.T pool released) ----
            with tc.tile_pool(name="s7", bufs=1) as s7, \
                 tc.tile_pool(name="st7", bufs=1) as st7, \
                 tc.tile_pool(name="ps7", bufs=8, space="PSUM") as ps7:
                zTf_sb = s7.tile([128, N], BF16)  # hi/lo split, new-j order
                for h in range(2):
                    for r in range(N_CORES):
                        eng = nc.sync if r % 2 == 0 else nc.gpsimd
                        eng.dma_start(
                            out=zTf_sb[:, h * 4096 + r * HP:h * 4096 + (r + 1) * HP],
                            in_=ag3_out[h][r * 128:(r + 1) * 128, :])
                zTl_sb = s7.tile([128, P], BF16, tag="zTl")
                for h in range(2):
                    nc.sync.dma_start(out=zTl_sb[:, h * HP:(h + 1) * HP],
                                      in_=ag3_in[h][:])
                nq = [0]

                def adj_dma(dst, src_tile):
                    eng = nc.sync if nq[0] % 2 == 0 else nc.gpsimd
                    nq[0] += 1
                    eng.dma_start(out=dst, in_=src_tile)

                for it in range(PT):
                    st = st7.tile([128, N_CORES, 2, HP], BF16, tag="adj_st",
                                  bufs=6)
                    if it == 0:
                        # h-major so nothing touches the second z gather
                        # until AG3b lands
                        order = [(h, r) for h in range(2)
                                 for r in range(N_CORES)]
                    else:
                        order = [(h, r) for r in range(N_CORES)
                                 for h in range(2)]
                    for n, (h, r) in enumerate(order):
                        jb = h * 8 + r
                        aps = ps7.tile([128, HP], F32, tag="aps")
                        nc.tensor.matmul(
                            aps[:],
                            zTl_sb[:, it * 128:(it + 1) * 128],
                            zTf_sb[:, jb * HP:(jb + 1) * HP],
                            start=True, stop=True)
                        if n % 2 == 0:
                            nc.vector.tensor_copy(st[:, r, h, :], aps[:])
                        else:
                            nc.scalar.copy(st[:, r, h, :], aps[:])
                        if it > 0 and n == 7:
                            adj_dma(adj_out_r[:, it, 0:4, :, :], st[:, 0:4])
                    if it == 0:
                        adj_dma(adj_out_r[:, it, :, :, :], st[:])
                    elif it < PT - 1:
                        adj_dma(adj_out_r[:, it, 4:8, :, :], st[:, 4:8])
                    else:
                        # final i-tile: smaller trailing transfers so the
                        # end-of-kernel barrier waits on less in-flight data
                        adj_dma(adj_out_r[:, it, 4:6, :, :], st[:, 4:6])
                        adj_dma(adj_out_r[:, it, 6:8, :, :], st[:, 6:8])

    nc.compile()
    return nc


def _get_nc():
    if "nc" not in _NC_CACHE:
        _NC_CACHE["nc"] = _build()
    return _NC_CACHE["nc"]


def _prep_in_maps(inputs):
    x = np.asarray(inputs["x"], dtype=np.float32)
    edge_src = np.asarray(inputs["edge_src"], dtype=np.int64)
    edge_dst = np.asarray(inputs["edge_dst"], dtype=np.int64)
    edge_w = np.asarray(inputs["edge_w"], dtype=np.float32)
    eps = np.asarray(inputs["eps"], dtype=np.float32)
    W1 = np.asarray(inputs["W1"], dtype=np.float32)
    b1 = np.asarray(inputs["b1"], dtype=np.float32)
    Wmu = np.asarray(inputs["Wmu"], dtype=np.float32)
    bmu = np.asarray(inputs["bmu"], dtype=np.float32)
    Wls = np.asarray(inputs["Wls"], dtype=np.float32)
    bls = np.asarray(inputs["bls"], dtype=np.float32)

    A = np.zeros((N, N), dtype=np.float32)
    np.add.at(A, (edge_src, edge_dst), edge_w)

    w1_h = np.ascontiguousarray(W1.astype(np.float16))
    wml_h = np.ascontiguousarray(
        np.concatenate([Wmu, Wls], axis=1).astype(np.float16))
    b1_h = np.ascontiguousarray(b1.reshape(1, -1).astype(np.float16))
    bml_h = np.ascontiguousarray(
        np.concatenate([bmu, bls]).reshape(1, -1).astype(np.float16))
    ident = np.eye(128, dtype=np.float32)

    in_maps = []
    for c in range(N_CORES):
        rows = slice(c * P, (c + 1) * P)
        at_c = A[rows].T[_PERM]  # [N, P], contraction axis in gather order
        in_maps.append({
            "xT": np.ascontiguousarray(x[rows].T.astype(np.float16)),
            "w1": w1_h,
            "b1": b1_h,
            "wml": wml_h,
            "bml": bml_h,
            "at": np.ascontiguousarray(at_c.astype(np.float16)),
            "epsT": np.ascontiguousarray(eps[rows].T),
            "ident": ident,
        })
    return in_maps


def _run(in_maps, trace=False):
    nc = _get_nc()
    kw = {}
    if trace:
        kw["trace"] = True
    return run_bass_kernel_spmd(nc, in_maps, core_ids=list(range(N_CORES)), **kw)


def kernel(_trace=False, **inputs):
    in_maps = _prep_in_maps(inputs)
    res = _run(in_maps, trace=_trace)
    _NC_CACHE["last_exec_ns"] = res.exec_time_ns
    z = np.concatenate([res.results[c]["z"] for c in range(N_CORES)], axis=0)
    adj = np.concatenate(
        [res.results[c]["adj"].astype(np.float32) for c in range(N_CORES)],
        axis=0)
    mu = np.concatenate([res.results[c]["mu"] for c in range(N_CORES)], axis=0)
    ls = np.concatenate([res.results[c]["ls"] for c in range(N_CORES)], axis=0)
    return z, adj, mu, ls


# revision 27
# speedup vs baseline: 1.1422x; 1.1422x over previous
"""Distributed Trainium2 kernel for the AdaGCL VGAE view (8 NeuronCores).

Computation (reference):
    h      = tanh(spmm(A, x @ W1 + b1))
    mu     = spmm(A, h @ Wmu + bmu)
    logstd = spmm(A, h @ Wls + bls)
    z      = mu + eps * exp(logstd)
    adj    = z @ z.T
Returns (z, adj, mu, logstd).

Strategy: 1D row partition of nodes over 8 cores. The sparse adjacency
(E=262144 edges over N=8192 nodes) is densified on the host into per-core
A.T column blocks stored fp16 (resident in SBUF); spmm becomes TensorE
matmuls with fp32 PSUM accumulation. bf16 is numerically insufficient here
(logstd spans +-19, exp() amplifies absolute error; fp16's 11-bit mantissa
keeps the relative error ~2e-3). z stays fp32 and z @ z.T runs as a
bf16 hi/lo split (hi = bf16(z), lo = bf16(z - hi) stacked on the K axis)
so one K=128 bf16 matmul yields ~fp32 precision at full PE rate.

Communication: the three AllGathers
(pre-activations fp16, head pre-activations fp16, z^T fp32) are each split
into two half-node collectives; the contraction (j) axis is permuted
host-side into gather order (half major, then rank, then row) so compute
on the first half overlaps the second half's wire time.
"""

import sys
import numpy as np

sys.path.insert(0, "/opt/trn_rl_repo")

from concourse import bacc, mybir, tile  # noqa: E402
from concourse.bass_utils import run_bass_kernel_spmd  # noqa: E402

N = 8192
E = 262144
IN_DIM, HID_DIM, Z_DIM = 512, 256, 64
N_CORES = 8
P = N // N_CORES          # 1024 rows per core
HP = P // 2               # 512 rows per gather half
NT = N // 128             # 64 j-tiles of 128 nodes
PT = P // 128             # 8 i-tiles per core

F16 = mybir.dt.float16
F32 = mybir.dt.float32
BF16 = mybir.dt.bfloat16

_NC_CACHE = {}

# new-j -> old-j permutation of the contraction axis: jj = (h, r, o)
# with h in {0,1} half, r rank, o in [0, 512) -> old = r*1024 + h*512 + o.
_PERM = np.concatenate([
    r * P + h * HP + np.arange(HP)
    for h in range(2) for r in range(N_CORES)
])


def _build():
    nc = bacc.Bacc("TRN2", target_bir_lowering=False, debug=False,
                   num_devices=N_CORES)

    # ---- per-core external inputs ----
    xT = nc.dram_tensor("xT", [IN_DIM, P], F16, kind="ExternalInput")
    w1 = nc.dram_tensor("w1", [IN_DIM, HID_DIM], F16, kind="ExternalInput")
    b1 = nc.dram_tensor("b1", [1, HID_DIM], F16, kind="ExternalInput")
    wml = nc.dram_tensor("wml", [HID_DIM, 2 * Z_DIM], F16, kind="ExternalInput")
    bml = nc.dram_tensor("bml", [1, 2 * Z_DIM], F16, kind="ExternalInput")
    at = nc.dram_tensor("at", [N, P], F16, kind="ExternalInput")  # A.T, j perm'd
    epsT = nc.dram_tensor("epsT", [Z_DIM, P], F32, kind="ExternalInput")
    ident = nc.dram_tensor("ident", [128, 128], F32, kind="ExternalInput")

    # ---- per-core external outputs (row shards) ----
    z_out = nc.dram_tensor("z", [P, Z_DIM], F32, kind="ExternalOutput")
    adj_out = nc.dram_tensor("adj", [P, N], BF16, kind="ExternalOutput")
    mu_out = nc.dram_tensor("mu", [P, Z_DIM], F32, kind="ExternalOutput")
    ls_out = nc.dram_tensor("ls", [P, Z_DIM], F32, kind="ExternalOutput")

    # ---- collective bounce buffers ----
    ag1_in = [nc.dram_tensor(f"ag1_in{h}", [HP, HID_DIM], F16) for h in range(2)]
    ag1_out = [nc.dram_tensor(f"ag1_out{h}", [N_CORES * HP, HID_DIM], F16,
                              addr_space="Shared") for h in range(2)]
    ag2_in = [nc.dram_tensor(f"ag2_in{h}", [HP, 2 * Z_DIM], F16) for h in range(2)]
    ag2_out = [nc.dram_tensor(f"ag2_out{h}", [N_CORES * HP, 2 * Z_DIM], F16,
                              addr_space="Shared") for h in range(2)]
    # z^T gathered as bf16 hi/lo split stacked on the partition axis:
    # rows 0:64 = bf16(z^T), rows 64:128 = bf16(z^T - hi). One K=128 bf16
    # matmul then computes (hi+lo) @ (hi+lo)^T with fp32 accumulation --
    # full-rate PE at ~fp32 precision (rel err ~2^-16).
    ag3_in = [nc.dram_tensor(f"ag3_in{h}", [128, HP], BF16) for h in range(2)]
    ag3_out = [nc.dram_tensor(f"ag3_out{h}", [N_CORES * 128, HP], BF16,
                              addr_space="Shared") for h in range(2)]

    groups = [list(range(N_CORES))]

    def allgather(src, dst):
        nc.gpsimd.collective_compute(
            "AllGather", mybir.AluOpType.bypass, replica_groups=groups,
            ins=[src.ap().opt()], outs=[dst.ap().opt()])

    at_r = at.ap().rearrange("(t p) i -> p t i", p=128)        # [128, 64, 1024]
    xT_r = xT.ap().rearrange("(t p) i -> p t i", p=128)        # [128, 4, 1024]
    w1_r = w1.ap().rearrange("(t p) f -> p t f", p=128)        # [128, 4, 256]
    wml_r = wml.ap().rearrange("(t p) f -> p t f", p=128)      # [128, 2, 128]
    ag1_in_r = [a.ap().rearrange("(t p) f -> p t f", p=128) for a in ag1_in]
    ag2_in_r = [a.ap().rearrange("(t p) f -> p t f", p=128) for a in ag2_in]
    ag1_out_r = [a.ap().rearrange("(t p) f -> p t f", p=128) for a in ag1_out]
    ag2_out_r = [a.ap().rearrange("(t p) f -> p t f", p=128) for a in ag2_out]
    # adj columns in (rank, half, offset) original order vs new-j order
    adj_out_r = adj_out.ap().rearrange("(t p) (r h o) -> p t r h o",
                                       p=128, h=2, o=HP)
    z_out_r = z_out.ap().rearrange("(t p) f -> p t f", p=128)
    mu_out_r = mu_out.ap().rearrange("(t p) f -> p t f", p=128)
    ls_out_r = ls_out.ap().rearrange("(t p) f -> p t f", p=128)

    with tile.TileContext(nc) as tc:
        with tc.tile_pool(name="work", bufs=1) as work:
            ones = work.tile([1, 128], F16, tag="ones")
            nc.vector.memset(ones[:], 1.0)
            ident_sb = work.tile([128, 128], F32, tag="ident")
            nc.scalar.dma_start(out=ident_sb[:], in_=ident[:])

            with tc.tile_pool(name="res", bufs=1) as res:
                # A.T resident for both spmm passes: [128, 64, 1024] fp16.
                # gpsimd (SWDGE) queue so the sync queue serves the
                # latency-critical stage-1 inputs first.
                at_sb = res.tile([128, NT, P], F16)
                for c in range(8):
                    nc.gpsimd.dma_start(out=at_sb[:, 8 * c:8 * c + 8, :],
                                        in_=at_r[:, 8 * c:8 * c + 8, :])

                # ---- stage 1: pre_h = x @ W1 + b1 (per-core rows) ----
                with tc.tile_pool(name="s1", bufs=1) as s1, \
                     tc.tile_pool(name="ps1", bufs=4, space="PSUM") as ps1:
                    xT_sb = s1.tile([128, IN_DIM // 128, P], F16)
                    w1_sb = s1.tile([128, IN_DIM // 128, HID_DIM], F16)
                    b1_sb = s1.tile([1, HID_DIM], F16)
                    nc.sync.dma_start(out=xT_sb[:], in_=xT_r[:])
                    nc.sync.dma_start(out=w1_sb[:], in_=w1_r[:])
                    nc.sync.dma_start(out=b1_sb[:], in_=b1[:])
                    for h in range(2):
                        for lt in range(PT // 2):
                            it = h * (PT // 2) + lt
                            ph = ps1.tile([128, HID_DIM], F32)
                            for kt in range(IN_DIM // 128):
                                nc.tensor.matmul(
                                    ph[:],
                                    xT_sb[:, kt, it * 128:(it + 1) * 128],
                                    w1_sb[:, kt, :],
                                    start=(kt == 0), stop=False)
                            nc.tensor.matmul(ph[:], ones[:], b1_sb[:],
                                             start=False, stop=True)
                            ph_sb = s1.tile([128, HID_DIM], F16, tag="ph_sb",
                                            bufs=4)
                            if it % 2 == 0:
                                nc.scalar.copy(ph_sb[:], ph[:])
                            else:
                                nc.vector.tensor_copy(ph_sb[:], ph[:])
                            nc.sync.dma_start(out=ag1_in_r[h][:, lt, :],
                                              in_=ph_sb[:])
                        allgather(ag1_in[h], ag1_out[h])

                # ---- stage 3 + heads, pipelined by output half ----
                # The A @ pre_h accumulation runs once per i-half; as soon as
                # half 0's tanh + head projections are out, AG2a flies while
                # the PE grinds half 1.
                hT_sb = work.tile([128, HID_DIM // 128, P], F16, tag="hT")
                with tc.tile_pool(name="s3", bufs=1) as s3, \
                     tc.tile_pool(name="ps3", bufs=1, space="PSUM") as ps3, \
                     tc.tile_pool(name="ps4", bufs=4, space="PSUM") as ps4:
                    preh_sb = s3.tile([128, NT, HID_DIM], F16)
                    wml_sb = s3.tile([128, HID_DIM // 128, 2 * Z_DIM], F16)
                    bml_sb = s3.tile([1, 2 * Z_DIM], F16)
                    nc.sync.dma_start(out=wml_sb[:], in_=wml_r[:])
                    nc.sync.dma_start(out=bml_sb[:], in_=bml[:])
                    for h in range(2):
                        for c in range(4):
                            eng = nc.sync if c % 2 == 0 else nc.gpsimd
                            o = h * 32 + 8 * c
                            eng.dma_start(out=preh_sb[:, o:o + 8, :],
                                          in_=ag1_out_r[h][:, 8 * c:8 * c + 8, :])
                    hps = [[ps3.tile([128, 512], F32, name=f"hps{ih}{i}",
                                     tag=f"hps{ih}{i}") for i in range(2)]
                           for ih in range(2)]
                    # j-tiles 0..31 for all four groups first (AG1b is still
                    # in flight); then finish each i-half's j 32..63 and fire
                    # its downstream work immediately.
                    for jt in range(NT // 2):
                        for ft in range(2):
                            for ih in range(2):
                                nc.tensor.matmul(
                                    hps[ih][ft][:],
                                    preh_sb[:, jt, ft * 128:(ft + 1) * 128],
                                    at_sb[:, jt, ih * 512:(ih + 1) * 512],
                                    start=(jt == 0), stop=False)
                    for ih in range(2):
                        for jt in range(NT // 2, NT):
                            for ft in range(2):
                                nc.tensor.matmul(
                                    hps[ih][ft][:],
                                    preh_sb[:, jt, ft * 128:(ft + 1) * 128],
                                    at_sb[:, jt, ih * 512:(ih + 1) * 512],
                                    start=False, stop=(jt == NT - 1))
                        for ft in range(2):
                            nc.scalar.activation(
                                hT_sb[:, ft, ih * 512:(ih + 1) * 512],
                                hps[ih][ft][:],
                                mybir.ActivationFunctionType.Tanh)
                        for lt in range(PT // 2):
                            it = ih * (PT // 2) + lt
                            mlp = ps4.tile([128, 2 * Z_DIM], F32)
                            for ft in range(HID_DIM // 128):
                                nc.tensor.matmul(
                                    mlp[:],
                                    hT_sb[:, ft, it * 128:(it + 1) * 128],
                                    wml_sb[:, ft, :],
                                    start=(ft == 0), stop=False)
                            nc.tensor.matmul(mlp[:], ones[:], bml_sb[:],
                                             start=False, stop=True)
                            ml_sb = s3.tile([128, 2 * Z_DIM], F16, tag="ml_sb",
                                            bufs=4)
                            if it % 2 == 0:
                                nc.scalar.copy(ml_sb[:], mlp[:])
                            else:
                                nc.vector.tensor_copy(ml_sb[:], mlp[:])
                            nc.sync.dma_start(out=ag2_in_r[ih][:, lt, :],
                                              in_=ml_sb[:])
                        allgather(ag2_in[ih], ag2_out[ih])

                # ---- stage 5 + reparameterize, pipelined by i-half ----
                # Each i-half: accumulate (A @ ml).T, then immediately build
                # z^T hi/lo for that half and fire its AllGather while the
                # PE grinds the other half.
                mlT_sb = work.tile([128, P], F32, tag="mlT")
                zT_sb = work.tile([Z_DIM, P], F32, tag="zT")
                with tc.tile_pool(name="s5", bufs=1) as s5, \
                     tc.tile_pool(name="ps5", bufs=1, space="PSUM") as ps5:
                    mlf_sb = s5.tile([128, NT, 2 * Z_DIM], F16)
                    for h in range(2):
                        for c in range(2):
                            eng = nc.sync if c % 2 == 0 else nc.gpsimd
                            o = h * 32 + 16 * c
                            eng.dma_start(
                                out=mlf_sb[:, o:o + 16, :],
                                in_=ag2_out_r[h][:, 16 * c:16 * c + 16, :])
                    epsT_sb = s5.tile([Z_DIM, P], F32, tag="epsT")
                    nc.sync.dma_start(out=epsT_sb[:], in_=epsT[:])
                    expT_hi = s5.tile([128, P], F32, tag="expT_hi")
                    expT_lo = s5.tile([Z_DIM, P], F32, tag="expT_lo")
                    zhi = s5.tile([Z_DIM, P], BF16, tag="zhi")
                    zlo = s5.tile([Z_DIM, P], BF16, tag="zlo")
                    mps = [ps5.tile([128, 512], F32, name=f"mps{ih}",
                                    tag=f"mps{ih}") for ih in range(2)]
                    for jt in range(NT // 2):
                        for ih in range(2):
                            nc.tensor.matmul(
                                mps[ih][:],
                                mlf_sb[:, jt, :],
                                at_sb[:, jt, ih * 512:(ih + 1) * 512],
                                start=(jt == 0), stop=False)
                    for ih in range(2):
                        for jt in range(NT // 2, NT):
                            nc.tensor.matmul(
                                mps[ih][:],
                                mlf_sb[:, jt, :],
                                at_sb[:, jt, ih * 512:(ih + 1) * 512],
                                start=False, stop=(jt == NT - 1))
                        sl = slice(ih * HP, (ih + 1) * HP)
                        nc.scalar.copy(mlT_sb[:, sl], mps[ih][:])
                        nc.scalar.activation(expT_hi[64:128, sl],
                                             mlT_sb[64:128, sl],
                                             mybir.ActivationFunctionType.Exp)
                        nc.sync.dma_start(out=expT_lo[:, sl],
                                          in_=expT_hi[64:128, sl])
                        nc.vector.tensor_mul(zT_sb[:, sl], epsT_sb[:, sl],
                                             expT_lo[:, sl])
                        nc.vector.tensor_add(zT_sb[:, sl], zT_sb[:, sl],
                                             mlT_sb[0:64, sl])
                        nc.vector.tensor_copy(zhi[:, sl], zT_sb[:, sl])
                        nc.vector.tensor_sub(zlo[:, sl], zT_sb[:, sl],
                                             zhi[:, sl])
                        nc.sync.dma_start(out=ag3_in[ih][0:Z_DIM, :],
                                          in_=zhi[:, sl])
                        nc.sync.dma_start(out=ag3_in[ih][Z_DIM:128, :],
                                          in_=zlo[:, sl])
                        allgather(ag3_in[ih], ag3_out[ih])

                    # natural-layout outputs mu / logstd / z via PE transposes
                    with tc.tile_pool(name="ps6", bufs=2, space="PSUM") as ps6:
                        mlnat = s5.tile([128, PT, 2 * Z_DIM], F32, tag="mlnat")
                        znat = s5.tile([128, PT, Z_DIM], F32, tag="znat")
                        for it in range(PT):
                            tp = ps6.tile([128, 128], F32, tag="tp")
                            nc.tensor.transpose(
                                tp[:], mlT_sb[:, it * 128:(it + 1) * 128],
                                ident_sb[:])
                            nc.vector.tensor_copy(mlnat[:, it, :], tp[:])
                            tz = ps6.tile([128, Z_DIM], F32, tag="tz")
                            nc.tensor.transpose(
                                tz[:], zT_sb[:, it * 128:(it + 1) * 128],
                                ident_sb[0:64, 0:64])
                            nc.vector.tensor_copy(znat[:, it, :], tz[:])
                        nc.sync.dma_start(out=mu_out_r[:],
                                          in_=mlnat[:, :, 0:Z_DIM])
                        nc.sync.dma_start(out=ls_out_r[:],
                                          in_=mlnat[:, :, Z_DIM:2 * Z_DIM])
                        nc.sync.dma_start(out=z_out_r[:], in_=znat[:])

            # ---- stage 7: adj = z_shard @ z_full.T (A.T pool released) ----
            with tc.tile_pool(name="s7", bufs=1) as s7, \
                 tc.tile_pool(name="st7", bufs=1) as st7, \
                 tc.tile_pool(name="ps7", bufs=8, space="PSUM") as ps7:
                zTf_sb = s7.tile([128, N], BF16)  # hi/lo split, new-j order
                for h in range(2):
                    for r in range(N_CORES):
                        eng = nc.sync if r % 2 == 0 else nc.gpsimd
                        eng.dma_start(
                            out=zTf_sb[:, h * 4096 + r * HP:h * 4096 + (r + 1) * HP],
                            in_=ag3_out[h][r * 128:(r + 1) * 128, :])
                zTl_sb = s7.tile([128, P], BF16, tag="zTl")
                for h in range(2):
                    nc.sync.dma_start(out=zTl_sb[:, h * HP:(h + 1) * HP],
                                      in_=ag3_in[h][:])
                nq = [0]

                def adj_dma(dst, src_tile):
                    eng = nc.sync if nq[0] % 2 == 0 else nc.gpsimd
                    nq[0] += 1
                    eng.dma_start(out=dst, in_=src_tile)

                for it in range(PT):
                    st = st7.tile([128, N_CORES, 2, HP], BF16, tag="adj_st",
                                  bufs=6)
                    if it == 0:
                        # h-major so nothing touches the second z gather
                        # until AG3b lands
                        order = [(h, r) for h in range(2)
                                 for r in range(N_CORES)]
                    else:
                        order = [(h, r) for r in range(N_CORES)
                                 for h in range(2)]
                    for n, (h, r) in enumerate(order):
                        jb = h * 8 + r
                        aps = ps7.tile([128, HP], F32, tag="aps")
                        nc.tensor.matmul(
                            aps[:],
                            zTl_sb[:, it * 128:(it + 1) * 128],
                            zTf_sb[:, jb * HP:(jb + 1) * HP],
                            start=True, stop=True)
                        if n % 2 == 0:
                            nc.vector.tensor_copy(st[:, r, h, :], aps[:])
                        else:
                            nc.scalar.copy(st[:, r, h, :], aps[:])
                        if it > 0 and n == 7:
                            adj_dma(adj_out_r[:, it, 0:4, :, :], st[:, 0:4])
                    if it == 0:
                        adj_dma(adj_out_r[:, it, :, :, :], st[:])
                    else:
                        adj_dma(adj_out_r[:, it, 4:8, :, :], st[:, 4:8])

    nc.compile()
    return nc


def _get_nc():
    if "nc" not in _NC_CACHE:
        _NC_CACHE["nc"] = _build()
    return _NC_CACHE["nc"]


def _prep_in_maps(inputs):
    x = np.asarray(inputs["x"], dtype=np.float32)
    edge_src = np.asarray(inputs["edge_src"], dtype=np.int64)
    edge_dst = np.asarray(inputs["edge_dst"], dtype=np.int64)
    edge_w = np.asarray(inputs["edge_w"], dtype=np.float32)
    eps = np.asarray(inputs["eps"], dtype=np.float32)
    W1 = np.asarray(inputs["W1"], dtype=np.float32)
    b1 = np.asarray(inputs["b1"], dtype=np.float32)
    Wmu = np.asarray(inputs["Wmu"], dtype=np.float32)
    bmu = np.asarray(inputs["bmu"], dtype=np.float32)
    Wls = np.asarray(inputs["Wls"], dtype=np.float32)
    bls = np.asarray(inputs["bls"], dtype=np.float32)

    A = np.zeros((N, N), dtype=np.float32)
    np.add.at(A, (edge_src, edge_dst), edge_w)

    w1_h = np.ascontiguousarray(W1.astype(np.float16))
    wml_h = np.ascontiguousarray(
        np.concatenate([Wmu, Wls], axis=1).astype(np.float16))
    b1_h = np.ascontiguousarray(b1.reshape(1, -1).astype(np.float16))
    bml_h = np.ascontiguousarray(
        np.concatenate([bmu, bls]).reshape(1, -1).astype(np.float16))
    ident = np.eye(128, dtype=np.float32)

    in_maps = []
    for c in range(N_CORES):
        rows = slice(c * P, (c + 1) * P)
        at_c = A[rows].T[_PERM]  # [N, P], contraction axis in gather order
        in_maps.append({
            "xT": np.ascontiguousarray(x[rows].T.astype(np.float16)),
            "w1": w1_h,
            "b1": b1_h,
            "wml": wml_h,
            "bml": bml_h,
            "at": np.ascontiguousarray(at_c.astype(np.float16)),
            "epsT": np.ascontiguousarray(eps[rows].T),
            "ident": ident,
        })
    return in_maps


def _run(in_maps, trace=False):
    nc = _get_nc()
    kw = {}
    if trace:
        kw["trace"] = True
    return run_bass_kernel_spmd(nc, in_maps, core_ids=list(range(N_CORES)), **kw)


def kernel(_trace=False, **inputs):
    in_maps = _prep_in_maps(inputs)
    res = _run(in_maps, trace=_trace)
    _NC_CACHE["last_exec_ns"] = res.exec_time_ns
    z = np.concatenate([res.results[c]["z"] for c in range(N_CORES)], axis=0)
    adj = np.concatenate(
        [res.results[c]["adj"].astype(np.float32) for c in range(N_CORES)],
        axis=0)
    mu = np.concatenate([res.results[c]["mu"] for c in range(N_CORES)], axis=0)
    ls = np.concatenate([res.results[c]["ls"] for c in range(N_CORES)], axis=0)
    return z, adj, mu, ls
